# Initial kernel scaffold
#
"""Trainium2 Bass kernel for the binarized MLP (BNN) problem.

Network (eval mode):
  h1 = sign(bn1(x @ sign(w1).T + b1))        x: [8192, 784]
  h2 = sign(bn2(h1 @ sign(w2).T + b2))       hidden: 6144
  h3 = sign(bn3(h2 @ sign(w3).T + b3))
  out = log_softmax(h3 @ w4.T + b4)          out: [8192, 10]
(clip(-1,1) before sign does not change sign, so it is dropped.)

Strategy:
  * Data-parallel over the batch: 8 cores x 1024 rows, no collectives.
  * All activations live transposed in SBUF as hT[H, B] so each layer's
    output feeds the next layer's matmul rhs directly (zero transposes).
  * BN + bias + clip + binarize folds to sign(h*s + c) with
    s = g*rsqrt(v+eps), c = (b - m)*s + be  -> one scalar-engine
    activation (Sign) per psum tile with per-partition scale/bias.
  * Layer 1 (real-valued x): x split into 2 fp16 planes (hi/lo) for fp32
    precision (residual 2^-23; the PE keeps fp16 denormals); the two
    planes are STACKED along the contraction dim (1568 rows -> 13
    k-tiles) so the padding waste is paid once, not per-plane.
  * Layers 2/3 (+-1 x +-1, contraction 6144): fp8e4m3 with DoubleRow
    perf mode - products and fp32 PSUM accumulation are exact.
  * Layer 4 is INTERLEAVED into layer 3's matmul stream as fp8
    DoubleRow pairs (same perf mode as the L3 chains, so no
    weight-path mode switches): w4.T is split into fp8 hi and residual-lo
    planes (both x 2^7) stacked at stationary columns 0-9 / 32-41; the
    j-th k-tile pair issues right after sign(h3[2j+1]) lands, riding
    along with zero PE stalls.  The tail adds the two psum slices and
    the transpose identity carries the 2^-7 rescale.
  * log_softmax tail: PE transposes [10,128] logit blocks back to
    [128,10] (batch-major, so the output DMA moves 40-byte rows, not
    4-byte elements), per-block reduce_max + shift, then ONE batched
    Exp, ONE batched per-block reduce_sum, ONE batched Ln, one
    broadcast subtract, one store.
"""

import numpy as np
import ml_dtypes

H = 6144
B_TOTAL = 8192
N_CORES = 8
B = B_TOTAL // N_CORES  # 1024 rows per core
K1 = 784
KT1 = 13  # ceil(2*784 / 128) stacked hi|lo k-tiles
K1S = KT1 * 128  # 1664
EPS = 1e-5
P = 128
M_TILES = H // P  # 48
NB = B // 512  # psum-width chunks per core

_BF16 = ml_dtypes.bfloat16
_FP8 = ml_dtypes.float8_e4m3
_FP16 = np.float16


def _binarize(w):
    return np.where(w >= 0, np.float32(1.0), np.float32(-1.0))


def _pack_weight(wb, kpad, dtype):
    """[Hout, K] matrix -> [Hout/128, 128, kpad/128, 128] tiles where
    pack[m, p, ko, j] = wb[m*128 + j, ko*128 + p] (lhsT layout)."""
    hout, k = wb.shape
    if k < kpad:
        wb = np.concatenate([wb, np.zeros((hout, kpad - k), np.float32)], axis=1)
    return np.ascontiguousarray(
        wb.reshape(hout // P, P, kpad // P, P).transpose(0, 3, 2, 1)
    ).astype(dtype)


def _pack_rhs(xc):
    """[B, K] -> [128, K/128, B] with pack[p, ko, b] = xc[b, ko*128+p]."""
    b, k = xc.shape
    return np.ascontiguousarray(xc.T.reshape(k // P, P, b).transpose(1, 0, 2))


def build_nc():
    """Build the (single-program, run-on-8-cores) Bass kernel."""
    import concourse.tile as tile
    import concourse.mybir as mybir
    from concourse import bacc
    from concourse.masks import make_identity

    af = mybir.ActivationFunctionType
    f32 = mybir.dt.float32
    bf16 = mybir.dt.bfloat16
    f16 = mybir.dt.float16
    f8 = mybir.dt.float8e4

    nc = bacc.Bacc(
        "TRN2",
        target_bir_lowering=False,
        debug=False,
        enable_asserts=False,
        num_devices=N_CORES,
    )

    t = {}
    t["xstk"] = nc.dram_tensor("xstk", [P, KT1, B], f16, kind="ExternalInput").ap()
    t["w1p"] = nc.dram_tensor(
        "w1p", [M_TILES, P, KT1, P], f16, kind="ExternalInput"
    ).ap()
    for nm in ("w2p", "w3p"):
        t[nm] = nc.dram_tensor(
            nm, [M_TILES, P, M_TILES, P], f8, kind="ExternalInput"
        ).ap()
    t["w4s"] = nc.dram_tensor(
        "w4s", [M_TILES // 2, P, 2, 48], f8, kind="ExternalInput"
    ).ap()
    t["b4t"] = nc.dram_tensor("b4t", [1, 48], bf16, kind="ExternalInput").ap()
    for i in (1, 2, 3):
        t[f"s{i}"] = nc.dram_tensor(f"s{i}", [P, M_TILES], f32, kind="ExternalInput").ap()
        t[f"c{i}"] = nc.dram_tensor(f"c{i}", [P, M_TILES], f32, kind="ExternalInput").ap()
    t["out"] = nc.dram_tensor("out", [B, 10], f32, kind="ExternalOutput").ap()

    from contextlib import ExitStack

    with tile.TileContext(nc) as tc, ExitStack() as ctx:
        consts = ctx.enter_context(tc.tile_pool(name="consts", bufs=1))
        xpool = ctx.enter_context(tc.tile_pool(name="x", bufs=1))
        hpool = ctx.enter_context(tc.tile_pool(name="h", bufs=2))
        w1pool = ctx.enter_context(tc.tile_pool(name="w1", bufs=3))
        wpool = ctx.enter_context(tc.tile_pool(name="w", bufs=4))
        pspool = ctx.enter_context(tc.tile_pool(name="ps", bufs=3, space="PSUM"))
        ps20pool = ctx.enter_context(tc.tile_pool(name="ps20", bufs=2, space="PSUM"))
        ps4pool = ctx.enter_context(tc.tile_pool(name="ps4", bufs=1, space="PSUM"))
        small = ctx.enter_context(tc.tile_pool(name="small", bufs=2))

        # ---- one-time loads ----
        # The first chain consumes xstk[k][:, 0:512] at 216ns/k, faster than
        # full-width chunks arrive; stream the n=0 column halves first (both
        # queues), then w1p[0], then the n=1 halves.  Consts go last.
        xs_t = xpool.tile([P, KT1, B], f16, tag="xstk")
        wt0 = w1pool.tile([P, KT1, P], f16, tag="w1")
        for k in range(KT1):
            (nc.sync if k % 2 == 0 else nc.gpsimd).dma_start(
                xs_t[:, k, 0:512], t["xstk"][:, k, 0:512]
            )
            (nc.gpsimd if k % 2 == 0 else nc.sync).dma_start(
                wt0[:, k, :], t["w1p"][0][:, k, :]
            )
        for k in range(KT1):
            (nc.gpsimd if k % 2 == 0 else nc.sync).dma_start(
                xs_t[:, k, 512:1024], t["xstk"][:, k, 512:1024]
            )
        bn = []
        for i in (1, 2, 3):
            s_t = consts.tile([P, M_TILES], f32, tag=f"s{i}")
            nc.gpsimd.dma_start(s_t[:], t[f"s{i}"][:])
            c_t = consts.tile([P, M_TILES], f32, tag=f"c{i}")
            nc.gpsimd.dma_start(c_t[:], t[f"c{i}"][:])
            bn.append((s_t, c_t))
        w4sb = consts.tile([P, M_TILES // 2, 2, 48], f8, tag="w4")
        nc.gpsimd.dma_start(w4sb[:], t["w4s"].rearrange("j p i c -> p j i c"))
        b4sb = consts.tile([1, 48], bf16, tag="b4")
        nc.gpsimd.dma_start(b4sb[:], t["b4t"][:])
        ident10 = consts.tile([10, 10], f32, tag="ident")
        make_identity(nc, ident10[:])
        ones1 = consts.tile([1, 512], f8, tag="ones1")
        nc.vector.memset(ones1[:], 1.0)


        # ---- layer 1: stacked hi|lo fp16 planes, K = 1664 ----
        # Prefetch layer 2's first weight tile so its chain starts clean.
        w2t0 = wpool.tile([P, M_TILES, P], f8, tag="w")
        nc.gpsimd.dma_start(w2t0[:], t["w2p"][0])
        s_t, c_t = bn[0]
        h1 = hpool.tile([P, M_TILES, B], f8, tag="h")
        for m in range(M_TILES):
            if m == 0:
                wt = wt0
            else:
                wt = w1pool.tile([P, KT1, P], f16, tag="w1")
                nc.sync.dma_start(wt[:], t["w1p"][m])
            for n in range(NB):
                ps = pspool.tile([P, 512], f32, tag="ps")
                for k in range(KT1):
                    nc.tensor.matmul(
                        ps[:],
                        wt[:, k, :],
                        xs_t[:, k, n * 512 : (n + 1) * 512],
                        start=(k == 0),
                        stop=(k == KT1 - 1),
                    )
                nc.scalar.activation(
                    h1[:, m, n * 512 : (n + 1) * 512],
                    ps[:],
                    af.Sign,
                    bias=c_t[:, m : m + 1],
                    scale=s_t[:, m : m + 1],
                )

        # ---- layers 2 and 3 (+ layer-4 chain interleaved into layer 3) ----
        hin = h1
        ps20 = None
        for li, wname in ((1, "w2p"), (2, "w3p")):
            s_t, c_t = bn[li]
            hout = hpool.tile([P, M_TILES, B], f8, tag="h")
            is_l3 = li == 2
            if is_l3:
                ps20 = []
                for ni in range(NB):
                    ps20_t = ps20pool.tile([64, 512], f32, tag="ps20")
                    ps20.append(ps20_t)
                    # open the accumulation with b4*2^5 broadcast into the hi
                    # rows via a rank-1 matmul (ones column vector rhs)
                    nc.tensor.matmul(
                        ps20_t[0:48, :], b4sb[:], ones1[:], start=True, stop=False
                    )

            def emit_l4(j, ni):
                nc.tensor.matmul(
                    ps20[ni][0:48, :],
                    w4sb[:, j, :, :],
                    hout[:, 2 * j : 2 * j + 2, ni * 512 : (ni + 1) * 512],
                    start=False,
                    stop=(j == M_TILES // 2 - 1),
                    perf_mode=mybir.MatmulPerfMode.DoubleRow,
                )

            for m in range(M_TILES):
                if li == 1 and m == 0:
                    wt = w2t0
                else:
                    wt = wpool.tile([P, M_TILES, P], f8, tag="w")
                    (nc.sync if m % 2 == 0 else nc.gpsimd).dma_start(wt[:], t[wname][m])
                for n in range(NB):
                    ps = pspool.tile([P, 512], f32, tag="ps")
                    for k2 in range(M_TILES // 2):
                        nc.tensor.matmul(
                            ps[:],
                            wt[:, 2 * k2 : 2 * k2 + 2, :],
                            hin[:, 2 * k2 : 2 * k2 + 2, n * 512 : (n + 1) * 512],
                            start=(k2 == 0),
                            stop=(k2 == M_TILES // 2 - 1),
                            perf_mode=mybir.MatmulPerfMode.DoubleRow,
                        )
                    nc.scalar.activation(
                        hout[:, m, n * 512 : (n + 1) * 512],
                        ps[:],
                        af.Sign,
                        bias=c_t[:, m : m + 1],
                        scale=s_t[:, m : m + 1],
                    )
                if is_l3 and m >= 2 and m % 2 == 0:
                    emit_l4((m - 2) // 2, 0)
                    emit_l4((m - 2) // 2, 1)
            hin = hout
        h3 = hin

        # ---- log_softmax tail ----
        # Warm the Exp/Ln activation tables while the last L4 matmuls run.
        warm = small.tile([1, 1], f32, tag="warm")
        nc.vector.memset(warm[:], 1.0)
        we = small.tile([1, 1], f32, tag="we")
        nc.scalar.activation(we[:], warm[:], af.Exp)
        # Ln warm-up on a [P,1] ones tile: ln(1)=0 feeds the rescale below as
        # a per-partition zero bias, making the Ln table load a hard
        # dependency that the scheduler must run before the batched Exp.
        warmp = small.tile([P, 1], f32, tag="warmp")
        nc.vector.memset(warmp[:], 1.0)
        wlp = small.tile([P, 1], f32, tag="wlp")
        nc.scalar.activation(wlp[:], warmp[:], af.Ln)
        BCH = B // P  # 8 output row-chunks per core
        # all 16 transposed logit blocks land in ONE psum bank so every
        # remaining reduction runs batched (one DVE op each, not 8)
        pst_all = ps4pool.tile([P, BCH * 10], f32, tag="pst_all")
        for n in range(NB):
            # final L4 pair for this chunk, then its transposes immediately
            # so chunk 0's work overlaps chunk 1's Sign latency.
            emit_l4(M_TILES // 2 - 1, n)
            lo_sb = small.tile([10, 512], f32, tag="lo_sb")
            nc.vector.tensor_copy(lo_sb[:], ps20[n][32:42, :])
            lgb = small.tile([10, 512], f32, tag="lgb")
            nc.vector.tensor_add(lgb[:], ps20[n][0:10, :], lo_sb[:])
            for bi in range(4):
                ci = n * 4 + bi
                nc.tensor.matmul(
                    pst_all[:, ci * 10 : (ci + 1) * 10],
                    lgb[:, bi * P : (bi + 1) * P],
                    ident10[:],
                    is_transpose=True,
                    start=(ci == 0),
                    stop=(ci == NB * 4 - 1),
                )
        # batched pass: max / shift+rescale / Exp / per-block sum / Ln /
        # broadcast-subtract / store
        pst_v = pst_all[:].rearrange("p (b o) -> p b o", o=10)
        nmx_all = small.tile([P, BCH], f32, tag="nmx_all")
        nc.vector.reduce_max(nmx_all[:], pst_v, axis=mybir.AxisListType.X, negate=True)
        otp_raw = small.tile([P, BCH, 10], f32, tag="otp_raw")
        nc.vector.tensor_add(
            otp_raw[:],
            pst_v,
            nmx_all[:].rearrange("p (b o) -> p b o", o=1).broadcast_to([P, BCH, 10]),
        )
        otp_all = small.tile([P, BCH, 10], f32, tag="otp_all")
        nc.vector.tensor_scalar(
            otp_all[:],
            otp_raw[:],
            2.0**-7,
            wlp[:],
            mybir.AluOpType.mult,
            mybir.AluOpType.add,
        )
        ex_all = small.tile([P, BCH, 10], f32, tag="ex_all")
        nc.scalar.activation(ex_all[:], otp_all[:], af.Exp)
        se_all = small.tile([P, BCH], f32, tag="se_all")
        nc.vector.reduce_sum(se_all[:], ex_all[:], axis=mybir.AxisListType.X)
        ls_all = small.tile([P, BCH], f32, tag="ls_all")
        nc.scalar.activation(ls_all[:], se_all[:], af.Ln)
        acc = small.tile([P, BCH, 10], f32, tag="acc")
        nc.vector.tensor_sub(
            acc[:],
            otp_all[:],
            ls_all[:].rearrange("p (b o) -> p b o", o=1).broadcast_to([P, BCH, 10]),
        )
        nc.sync.dma_start(t["out"].rearrange("(p j) n -> p j n", j=BCH), acc[:])

    nc.compile()
    return nc


def prepare_in_maps(inputs):
    """Host-side packing: binarize weights, fold BN, split/stack/shard x."""
    x = np.asarray(inputs["x"], np.float32).reshape(-1, K1)

    w1b = _binarize(np.asarray(inputs["w1"], np.float32))
    w1stk = np.concatenate([w1b, w1b], axis=1)  # hi|lo planes share weights
    w1p = _pack_weight(w1stk, K1S, _FP16)
    w2p = _pack_weight(_binarize(np.asarray(inputs["w2"], np.float32)), H, _FP8)
    w3p = _pack_weight(_binarize(np.asarray(inputs["w3"], np.float32)), H, _FP8)

    # w4 scaled fp8 hi/lo split, transposed to [K, 10], DoubleRow k-tile
    # pairs, hi|lo stacked at stationary columns 0-9 / 32-41.
    w4 = np.asarray(inputs["w4"], np.float32)
    b4 = np.asarray(inputs["b4"], np.float32)
    w4T = np.ascontiguousarray(w4.T)  # [6144, 10]
    w4hi8 = (w4T * np.float32(2.0**7)).astype(_FP8)
    w4lo8 = (w4T * np.float32(2.0**7) - w4hi8.astype(np.float32)).astype(_FP8)
    w4s = np.zeros((M_TILES // 2, P, 2, 48), _FP8)
    w4s[:, :, :, 0:10] = w4hi8.reshape(M_TILES // 2, 2, P, 10).transpose(0, 2, 1, 3)
    w4s[:, :, :, 32:42] = w4lo8.reshape(M_TILES // 2, 2, P, 10).transpose(0, 2, 1, 3)
    # b4 pre-scaled by 2^7 to match the w4 plane scale in the psum chain,
    # padded to the full 48 stationary columns so the rank-1 matmul opens
    # the whole accumulation group
    b4t = np.zeros((1, 48), _BF16)
    b4t[0, 0:10] = (b4 * np.float32(2.0**7)).astype(_BF16)

    sc = {}
    for i in (1, 2, 3):
        g = np.asarray(inputs[f"g{i}"], np.float32)
        be = np.asarray(inputs[f"be{i}"], np.float32)
        m = np.asarray(inputs[f"m{i}"], np.float32)
        v = np.asarray(inputs[f"v{i}"], np.float32)
        b = np.asarray(inputs[f"b{i}"], np.float32)
        s = g / np.sqrt(v + np.float32(EPS))
        c = (b - m) * s + be
        sc[f"s{i}"] = np.ascontiguousarray(s.reshape(M_TILES, P).T)
        sc[f"c{i}"] = np.ascontiguousarray(c.reshape(M_TILES, P).T)

    # x: 2-way fp16 split (PE keeps fp16 denormals), stacked hi|lo
    x_hi = x.astype(_FP16)
    x_lo = (x - x_hi.astype(np.float32)).astype(_FP16)

    in_maps = []
    for core in range(N_CORES):
        sl = slice(core * B, (core + 1) * B)
        im = {
            "w1p": w1p,
            "w2p": w2p,
            "w3p": w3p,
            "w4s": w4s,
            "b4t": b4t,
            **sc,
        }
        # batch permutation: kernel position n*512+bi*128+p computes the
        # row stored at out[p*8 + (n*4+bi)], making the output DMA write
        # 320B contiguous per partition instead of 8 scattered 40B rows
        kk = np.arange(B)
        perm = (kk % 128) * (B // P) + (kk // 512) * 4 + (kk % 512) // 128
        xc = np.zeros((B, K1S), _FP16)
        xc[:, :K1] = x_hi[sl][perm]
        xc[:, K1 : 2 * K1] = x_lo[sl][perm]
        im["xstk"] = _pack_rhs(xc)
        in_maps.append(im)
    return in_maps


_NC_CACHE = []


def kernel(**inputs):
    import time

    from concourse.bass_utils import run_bass_kernel_spmd

    if not _NC_CACHE:
        _NC_CACHE.append(build_nc())
    nc = _NC_CACHE[0]

    in_maps = prepare_in_maps(inputs)
    last_err = None
    for attempt in range(3):
        try:
            res = run_bass_kernel_spmd(nc, in_maps, core_ids=list(range(N_CORES)))
            return np.concatenate([r["out"] for r in res.results], axis=0)
        except Exception as e:  # transient device errors (e.g. NRT exec unit)
            last_err = e
            time.sleep(5 * (attempt + 1))
    raise last_err



# revision 1
# speedup vs baseline: 10.7552x; 10.7552x over previous
"""Trainium2 Bass kernel for the binarized MLP (BNN) problem.

Network (eval mode):
  h1 = sign(bn1(x @ sign(w1).T + b1))        x: [8192, 784]
  h2 = sign(bn2(h1 @ sign(w2).T + b2))       hidden: 6144
  h3 = sign(bn3(h2 @ sign(w3).T + b3))
  out = log_softmax(h3 @ w4.T + b4)          out: [8192, 10]
(clip(-1,1) before sign does not change sign, so it is dropped.)

Strategy:
  * Data-parallel over the batch: 8 cores x 1024 rows, no collectives.
  * All activations live transposed in SBUF as hT[H, B] so each layer's
    output feeds the next layer's matmul rhs directly (zero transposes).
  * BN + bias + clip + binarize folds to sign(h*s + c) with
    s = g*rsqrt(v+eps), c = (b - m)*s + be  -> one scalar-engine
    activation (Sign) per psum tile with per-partition scale/bias.
  * Layer 1 (real-valued x): x split into 2 fp16 planes (hi/lo) for fp32
    precision (residual 2^-23; the PE keeps fp16 denormals); the two
    planes are STACKED along the contraction dim (1568 rows -> 13
    k-tiles) so the padding waste is paid once, not per-plane.
  * Layers 2/3 (+-1 x +-1, contraction 6144): fp8e4m3 with DoubleRow
    perf mode - products and fp32 PSUM accumulation are exact.
  * Layer 4 is INTERLEAVED into layer 3's matmul stream as fp8
    DoubleRow pairs (same perf mode as the L3 chains, so no
    weight-path mode switches): w4.T is split into fp8 hi and residual-lo
    planes (both x 2^7) stacked at stationary columns 0-9 / 32-41; the
    j-th k-tile pair issues right after sign(h3[2j+1]) lands, riding
    along with zero PE stalls.  The tail adds the two psum slices and
    the transpose identity carries the 2^-7 rescale.
  * log_softmax tail: PE transposes [10,128] logit blocks back to
    [128,10] (batch-major, so the output DMA moves 40-byte rows, not
    4-byte elements), per-block reduce_max + shift, then ONE batched
    Exp, ONE batched per-block reduce_sum, ONE batched Ln, one
    broadcast subtract, one store.
"""

import numpy as np
import ml_dtypes

H = 6144
B_TOTAL = 8192
N_CORES = 8
B = B_TOTAL // N_CORES  # 1024 rows per core
K1 = 784
KT1 = 13  # ceil(2*784 / 128) stacked hi|lo k-tiles
K1S = KT1 * 128  # 1664
EPS = 1e-5
P = 128
M_TILES = H // P  # 48
NB = B // 512  # psum-width chunks per core

_BF16 = ml_dtypes.bfloat16
_FP8 = ml_dtypes.float8_e4m3
_FP16 = np.float16


def _binarize(w):
    return np.where(w >= 0, np.float32(1.0), np.float32(-1.0))


def _pack_weight(wb, kpad, dtype):
    """[Hout, K] matrix -> [Hout/128, 128, kpad/128, 128] tiles where
    pack[m, p, ko, j] = wb[m*128 + j, ko*128 + p] (lhsT layout)."""
    hout, k = wb.shape
    if k < kpad:
        wb = np.concatenate([wb, np.zeros((hout, kpad - k), np.float32)], axis=1)
    return np.ascontiguousarray(
        wb.reshape(hout // P, P, kpad // P, P).transpose(0, 3, 2, 1)
    ).astype(dtype)


def _pack_rhs(xc):
    """[B, K] -> [128, K/128, B] with pack[p, ko, b] = xc[b, ko*128+p]."""
    b, k = xc.shape
    return np.ascontiguousarray(xc.T.reshape(k // P, P, b).transpose(1, 0, 2))


def build_nc():
    """Build the (single-program, run-on-8-cores) Bass kernel."""
    import concourse.tile as tile
    import concourse.mybir as mybir
    from concourse import bacc
    from concourse.masks import make_identity

    af = mybir.ActivationFunctionType
    f32 = mybir.dt.float32
    bf16 = mybir.dt.bfloat16
    f16 = mybir.dt.float16
    f8 = mybir.dt.float8e4

    nc = bacc.Bacc(
        "TRN2",
        target_bir_lowering=False,
        debug=False,
        enable_asserts=False,
        num_devices=N_CORES,
    )

    t = {}
    t["xstk"] = nc.dram_tensor("xstk", [P, KT1, B], f16, kind="ExternalInput").ap()
    t["w1p"] = nc.dram_tensor(
        "w1p", [M_TILES, P, KT1, P], f16, kind="ExternalInput"
    ).ap()
    for nm in ("w2p", "w3p"):
        t[nm] = nc.dram_tensor(
            nm, [M_TILES, P, M_TILES, P], f8, kind="ExternalInput"
        ).ap()
    t["w4s"] = nc.dram_tensor(
        "w4s", [M_TILES // 2, P, 2, 48], f8, kind="ExternalInput"
    ).ap()
    t["b4t"] = nc.dram_tensor("b4t", [1, 48], bf16, kind="ExternalInput").ap()
    for i in (1, 2, 3):
        t[f"s{i}"] = nc.dram_tensor(f"s{i}", [P, M_TILES], f32, kind="ExternalInput").ap()
        t[f"c{i}"] = nc.dram_tensor(f"c{i}", [P, M_TILES], f32, kind="ExternalInput").ap()
    t["out"] = nc.dram_tensor("out", [B, 10], f32, kind="ExternalOutput").ap()

    from contextlib import ExitStack

    with tile.TileContext(nc) as tc, ExitStack() as ctx:
        consts = ctx.enter_context(tc.tile_pool(name="consts", bufs=1))
        xpool = ctx.enter_context(tc.tile_pool(name="x", bufs=1))
        hpool = ctx.enter_context(tc.tile_pool(name="h", bufs=2))
        w1pool = ctx.enter_context(tc.tile_pool(name="w1", bufs=3))
        wpool = ctx.enter_context(tc.tile_pool(name="w", bufs=4))
        pspool = ctx.enter_context(tc.tile_pool(name="ps", bufs=3, space="PSUM"))
        ps20pool = ctx.enter_context(tc.tile_pool(name="ps20", bufs=2, space="PSUM"))
        ps4pool = ctx.enter_context(tc.tile_pool(name="ps4", bufs=1, space="PSUM"))
        small = ctx.enter_context(tc.tile_pool(name="small", bufs=2))

        # ---- one-time loads ----
        # The first chain consumes xstk[k][:, 0:512] at 216ns/k, faster than
        # full-width chunks arrive; stream the n=0 column halves first (both
        # queues), then w1p[0], then the n=1 halves.  Consts go last.
        xs_t = xpool.tile([P, KT1, B], f16, tag="xstk")
        wt0 = w1pool.tile([P, KT1, P], f16, tag="w1")
        for k in range(KT1):
            (nc.sync if k % 2 == 0 else nc.gpsimd).dma_start(
                xs_t[:, k, 0:512], t["xstk"][:, k, 0:512]
            )
            (nc.gpsimd if k % 2 == 0 else nc.sync).dma_start(
                wt0[:, k, :], t["w1p"][0][:, k, :]
            )
        for k in range(KT1):
            (nc.gpsimd if k % 2 == 0 else nc.sync).dma_start(
                xs_t[:, k, 512:1024], t["xstk"][:, k, 512:1024]
            )
        bn = []
        for i in (1, 2, 3):
            s_t = consts.tile([P, M_TILES], f32, tag=f"s{i}")
            nc.gpsimd.dma_start(s_t[:], t[f"s{i}"][:])
            c_t = consts.tile([P, M_TILES], f32, tag=f"c{i}")
            nc.gpsimd.dma_start(c_t[:], t[f"c{i}"][:])
            bn.append((s_t, c_t))
        w4sb = consts.tile([P, M_TILES // 2, 2, 48], f8, tag="w4")
        nc.gpsimd.dma_start(w4sb[:], t["w4s"].rearrange("j p i c -> p j i c"))
        b4sb = consts.tile([1, 48], bf16, tag="b4")
        nc.gpsimd.dma_start(b4sb[:], t["b4t"][:])
        ident10 = consts.tile([10, 10], f32, tag="ident")
        make_identity(nc, ident10[:])
        ones1 = consts.tile([1, 512], f8, tag="ones1")
        nc.vector.memset(ones1[:], 1.0)


        # ---- layer 1: stacked hi|lo fp16 planes, K = 1664 ----
        # Prefetch layer 2's first weight tile so its chain starts clean.
        w2t0 = wpool.tile([P, M_TILES, P], f8, tag="w")
        nc.gpsimd.dma_start(w2t0[:], t["w2p"][0])
        s_t, c_t = bn[0]
        h1 = hpool.tile([P, M_TILES, B], f8, tag="h")
        for m in range(M_TILES):
            if m == 0:
                wt = wt0
            else:
                wt = w1pool.tile([P, KT1, P], f16, tag="w1")
                nc.sync.dma_start(wt[:], t["w1p"][m])
            for n in range(NB):
                ps = pspool.tile([P, 512], f32, tag="ps")
                for k in range(KT1):
                    nc.tensor.matmul(
                        ps[:],
                        wt[:, k, :],
                        xs_t[:, k, n * 512 : (n + 1) * 512],
                        start=(k == 0),
                        stop=(k == KT1 - 1),
                    )
                nc.scalar.activation(
                    h1[:, m, n * 512 : (n + 1) * 512],
                    ps[:],
                    af.Sign,
                    bias=c_t[:, m : m + 1],
                    scale=s_t[:, m : m + 1],
                )

        # ---- layers 2 and 3 (+ layer-4 chain interleaved into layer 3) ----
        hin = h1
        ps20 = None
        for li, wname in ((1, "w2p"), (2, "w3p")):
            s_t, c_t = bn[li]
            hout = hpool.tile([P, M_TILES, B], f8, tag="h")
            is_l3 = li == 2
            if is_l3:
                ps20 = []
                for ni in range(NB):
                    ps20_t = ps20pool.tile([64, 512], f32, tag="ps20")
                    ps20.append(ps20_t)
                    # open the accumulation with b4*2^5 broadcast into the hi
                    # rows via a rank-1 matmul (ones column vector rhs)
                    nc.tensor.matmul(
                        ps20_t[0:48, :], b4sb[:], ones1[:], start=True, stop=False
                    )

            def emit_l4(j, ni):
                nc.tensor.matmul(
                    ps20[ni][0:48, :],
                    w4sb[:, j, :, :],
                    hout[:, 2 * j : 2 * j + 2, ni * 512 : (ni + 1) * 512],
                    start=False,
                    stop=(j == M_TILES // 2 - 1),
                    perf_mode=mybir.MatmulPerfMode.DoubleRow,
                )

            for m in range(M_TILES):
                if li == 1 and m == 0:
                    wt = w2t0
                else:
                    wt = wpool.tile([P, M_TILES, P], f8, tag="w")
                    (nc.sync if m % 2 == 0 else nc.gpsimd).dma_start(wt[:], t[wname][m])
                for n in range(NB):
                    ps = pspool.tile([P, 512], f32, tag="ps")
                    for k2 in range(M_TILES // 2):
                        nc.tensor.matmul(
                            ps[:],
                            wt[:, 2 * k2 : 2 * k2 + 2, :],
                            hin[:, 2 * k2 : 2 * k2 + 2, n * 512 : (n + 1) * 512],
                            start=(k2 == 0),
                            stop=(k2 == M_TILES // 2 - 1),
                            perf_mode=mybir.MatmulPerfMode.DoubleRow,
                        )
                    nc.scalar.activation(
                        hout[:, m, n * 512 : (n + 1) * 512],
                        ps[:],
                        af.Sign,
                        bias=c_t[:, m : m + 1],
                        scale=s_t[:, m : m + 1],
                    )
                if is_l3 and m >= 2 and m % 2 == 0:
                    emit_l4((m - 2) // 2, 0)
                    emit_l4((m - 2) // 2, 1)
            hin = hout
        h3 = hin

        # ---- log_softmax tail ----
        # Warm the Exp/Ln activation tables while the last L4 matmuls run.
        warm = small.tile([1, 1], f32, tag="warm")
        nc.vector.memset(warm[:], 1.0)
        we = small.tile([1, 1], f32, tag="we")
        nc.scalar.activation(we[:], warm[:], af.Exp)
        # Ln warm-up on a [P,1] ones tile: ln(1)=0 feeds the rescale below as
        # a per-partition zero bias, making the Ln table load a hard
        # dependency that the scheduler must run before the batched Exp.
        warmp = small.tile([P, 1], f32, tag="warmp")
        nc.vector.memset(warmp[:], 1.0)
        wlp = small.tile([P, 1], f32, tag="wlp")
        nc.scalar.activation(wlp[:], warmp[:], af.Ln)
        BCH = B // P  # 8 output row-chunks per core
        # all 16 transposed logit blocks land in ONE psum bank so every
        # remaining reduction runs batched (one DVE op each, not 8)
        pst_all = ps4pool.tile([P, BCH * 10], f32, tag="pst_all")
        for n in range(NB):
            # final L4 pair for this chunk, then its transposes immediately
            # so chunk 0's work overlaps chunk 1's Sign latency.
            emit_l4(M_TILES // 2 - 1, n)
            lo_sb = small.tile([10, 512], f32, tag="lo_sb")
            nc.vector.tensor_copy(lo_sb[:], ps20[n][32:42, :])
            lgb = small.tile([10, 512], f32, tag="lgb")
            nc.vector.tensor_add(lgb[:], ps20[n][0:10, :], lo_sb[:])
            for bi in range(4):
                ci = n * 4 + bi
                nc.tensor.matmul(
                    pst_all[:, ci * 10 : (ci + 1) * 10],
                    lgb[:, bi * P : (bi + 1) * P],
                    ident10[:],
                    is_transpose=True,
                    start=(ci == 0),
                    stop=(ci == NB * 4 - 1),
                )
        # batched pass: max / shift+rescale / Exp / per-block sum / Ln /
        # broadcast-subtract / store
        pst_v = pst_all[:].rearrange("p (b o) -> p b o", o=10)
        nmx_all = small.tile([P, BCH], f32, tag="nmx_all")
        nc.vector.reduce_max(nmx_all[:], pst_v, axis=mybir.AxisListType.X, negate=True)
        otp_raw = small.tile([P, BCH, 10], f32, tag="otp_raw")
        nc.vector.tensor_add(
            otp_raw[:],
            pst_v,
            nmx_all[:].rearrange("p (b o) -> p b o", o=1).broadcast_to([P, BCH, 10]),
        )
        otp_all = small.tile([P, BCH, 10], f32, tag="otp_all")
        nc.vector.tensor_scalar(
            otp_all[:],
            otp_raw[:],
            2.0**-7,
            wlp[:],
            mybir.AluOpType.mult,
            mybir.AluOpType.add,
        )
        ex_all = small.tile([P, BCH, 10], f32, tag="ex_all")
        nc.scalar.activation(ex_all[:], otp_all[:], af.Exp)
        se_all = small.tile([P, BCH], f32, tag="se_all")
        nc.vector.reduce_sum(se_all[:], ex_all[:], axis=mybir.AxisListType.X)
        ls_all = small.tile([P, BCH], f32, tag="ls_all")
        nc.scalar.activation(ls_all[:], se_all[:], af.Ln)
        acc = small.tile([P, BCH, 10], f32, tag="acc")
        nc.vector.tensor_sub(
            acc[:],
            otp_all[:],
            ls_all[:].rearrange("p (b o) -> p b o", o=1).broadcast_to([P, BCH, 10]),
        )
        nc.sync.dma_start(t["out"].rearrange("(p j) n -> p j n", j=BCH), acc[:])

    nc.compile()
    return nc


def prepare_in_maps(inputs):
    """Host-side packing: binarize weights, fold BN, split/stack/shard x."""
    x = np.asarray(inputs["x"], np.float32).reshape(-1, K1)

    w1b = _binarize(np.asarray(inputs["w1"], np.float32))
    w1stk = np.concatenate([w1b, w1b], axis=1)  # hi|lo planes share weights
    w1p = _pack_weight(w1stk, K1S, _FP16)
    w2p = _pack_weight(_binarize(np.asarray(inputs["w2"], np.float32)), H, _FP8)
    w3p = _pack_weight(_binarize(np.asarray(inputs["w3"], np.float32)), H, _FP8)

    # w4 scaled fp8 hi/lo split, transposed to [K, 10], DoubleRow k-tile
    # pairs, hi|lo stacked at stationary columns 0-9 / 32-41.
    w4 = np.asarray(inputs["w4"], np.float32)
    b4 = np.asarray(inputs["b4"], np.float32)
    w4T = np.ascontiguousarray(w4.T)  # [6144, 10]
    w4hi8 = (w4T * np.float32(2.0**7)).astype(_FP8)
    w4lo8 = (w4T * np.float32(2.0**7) - w4hi8.astype(np.float32)).astype(_FP8)
    w4s = np.zeros((M_TILES // 2, P, 2, 48), _FP8)
    w4s[:, :, :, 0:10] = w4hi8.reshape(M_TILES // 2, 2, P, 10).transpose(0, 2, 1, 3)
    w4s[:, :, :, 32:42] = w4lo8.reshape(M_TILES // 2, 2, P, 10).transpose(0, 2, 1, 3)
    # b4 pre-scaled by 2^7 to match the w4 plane scale in the psum chain,
    # padded to the full 48 stationary columns so the rank-1 matmul opens
    # the whole accumulation group
    b4t = np.zeros((1, 48), _BF16)
    b4t[0, 0:10] = (b4 * np.float32(2.0**7)).astype(_BF16)

    sc = {}
    for i in (1, 2, 3):
        g = np.asarray(inputs[f"g{i}"], np.float32)
        be = np.asarray(inputs[f"be{i}"], np.float32)
        m = np.asarray(inputs[f"m{i}"], np.float32)
        v = np.asarray(inputs[f"v{i}"], np.float32)
        b = np.asarray(inputs[f"b{i}"], np.float32)
        s = g / np.sqrt(v + np.float32(EPS))
        c = (b - m) * s + be
        sc[f"s{i}"] = np.ascontiguousarray(s.reshape(M_TILES, P).T)
        sc[f"c{i}"] = np.ascontiguousarray(c.reshape(M_TILES, P).T)

    # x: 2-way fp16 split (PE keeps fp16 denormals), stacked hi|lo
    x_hi = x.astype(_FP16)
    x_lo = (x - x_hi.astype(np.float32)).astype(_FP16)

    in_maps = []
    for core in range(N_CORES):
        sl = slice(core * B, (core + 1) * B)
        im = {
            "w1p": w1p,
            "w2p": w2p,
            "w3p": w3p,
            "w4s": w4s,
            "b4t": b4t,
            **sc,
        }
        # batch permutation: kernel position n*512+bi*128+p computes the
        # row stored at out[p*8 + (n*4+bi)], making the output DMA write
        # 320B contiguous per partition instead of 8 scattered 40B rows
        kk = np.arange(B)
        perm = (kk % 128) * (B // P) + (kk // 512) * 4 + (kk % 512) // 128
        xc = np.zeros((B, K1S), _FP16)
        xc[:, :K1] = x_hi[sl][perm]
        xc[:, K1 : 2 * K1] = x_lo[sl][perm]
        im["xstk"] = _pack_rhs(xc)
        in_maps.append(im)
    return in_maps


_NC_CACHE = []


def kernel(**inputs):
    import time

    from concourse.bass_utils import run_bass_kernel_spmd

    if not _NC_CACHE:
        _NC_CACHE.append(build_nc())
    nc = _NC_CACHE[0]

    in_maps = prepare_in_maps(inputs)
    last_err = None
    for attempt in range(3):
        try:
            res = run_bass_kernel_spmd(nc, in_maps, core_ids=list(range(N_CORES)))
            return np.concatenate([r["out"] for r in res.results], axis=0)
        except Exception as e:  # transient device errors (e.g. NRT exec unit)
            last_err = e
            time.sleep(5 * (attempt + 1))
    raise last_err

